# revision 17
# baseline (speedup 1.0000x reference)
"""Trainium2 Bass kernel for causal multi-head attention with RoPE.

Model: B=2, T=2048, C=2048, H=16 heads, D=128 head_dim.
  qkv = x @ w_qkv ; q,k rotary-embedded ; causal softmax attention ; out @ w_out.

Sharding: tensor-parallel over heads. 16 heads / 8 cores = 2 heads per core.
Each core gets w_qkv columns and w_out rows for its 2 heads, computes a full
(B*T, C) partial output projection, and the host sums the 8 partials.

Per-core dataflow (matmul operands in float32r = full-rate rounded fp32):
  - x is fed pre-transposed (xT, [C, B*T]) so the C contraction sits on
    partitions.  qT/kT come out of the projection directly in [D, T] layout
    (D on partitions), v in natural [T, D] layout.
  - RoPE on DVE in [D, T] layout: rot_half is a partition-half swap done with
    two ACT copies, the sign folded into the sin table host-side.
  - scores computed transposed (sT[tk, tq] = kT.T @ qT), exp on ACT with the
    1/sqrt(D) scale folded in; causal mask additive on diagonal squares only,
    fully-masked column ranges zeroed via Copy(scale=0).
  - out_un[d, tq] accumulates v.T @ expT on PE; colsum via ones-column matmul.
  - softmax normalization deferred to the out_un PSUM->SBUF copy, multiplying
    by a partition-broadcast reciprocal colsum row (gpsimd broadcast).
  - output projection contracts the 2 local heads, interleaved per tq block,
    streamed straight to HBM.
"""

import numpy as np

import concourse.bass as bass
import concourse.tile as tile
import concourse.mybir as mybir
from concourse import bacc
from concourse.bass import ds
from concourse.bass_utils import run_bass_kernel_spmd

B, T, C, H, D = 2, 2048, 2048, 16, 128
NCORES = 8
HPC = H // NCORES  # heads per core = 2
S = B * T  # 4096 tokens
NBLK = T // 512  # 4 tq/tok blocks per batch
NCT = C // 128  # 16 contraction tiles for the qkv projection
NTK = T // 128  # 16 tk tiles per batch
F32 = mybir.dt.float32
F32R = mybir.dt.float32r
EXP_SCALE = float(D) ** -0.5
NEG = -1.0e30

_CACHE = {}


def build_nc():
    nc = bacc.Bacc("TRN2", target_bir_lowering=False, debug=False, num_devices=NCORES)

    xt_d = nc.dram_tensor("xt", [C, S], F32R, kind="ExternalInput").ap()
    wqkv_d = nc.dram_tensor("wqkv", [C, 6 * D], F32R, kind="ExternalInput").ap()
    wout_d = nc.dram_tensor("wout", [HPC * D, C], F32R, kind="ExternalInput").ap()
    cos_d = nc.dram_tensor("cos2t", [D, T], F32, kind="ExternalInput").ap()
    sin_d = nc.dram_tensor("sin2t", [D, T], F32, kind="ExternalInput").ap()
    mask_d = nc.dram_tensor("maskadd", [128, 128], F32R, kind="ExternalInput").ap()
    ones_d = nc.dram_tensor("ones_in", [128, 1], F32R, kind="ExternalInput").ap()
    y_d = nc.dram_tensor("y", [S, C], F32, kind="ExternalOutput").ap()

    xt_t = xt_d.rearrange("(ct p) s -> p ct s", p=128)  # [128, 16, 4096]
    wqkv_t = wqkv_d.rearrange("(ct p) n -> p ct n", p=128)  # [128, 16, 768]
    wout_t = wout_d.rearrange("(h p) n -> p h n", p=128)  # [128, 2, 2048]

    Exp = mybir.ActivationFunctionType.Exp
    Copy = mybir.ActivationFunctionType.Copy

    with tile.TileContext(nc) as tc:
        with (
            tc.tile_pool(name="s1", bufs=1) as s1,
            tc.tile_pool(name="s2", bufs=2) as s2,
            tc.tile_pool(name="se", bufs=5) as se,
            tc.tile_pool(name="sy", bufs=10) as sy,
            tc.tile_pool(name="sou", bufs=4) as sou,
            tc.tile_pool(name="ps2", bufs=2, space="PSUM") as ps2,
            tc.tile_pool(name="ps1", bufs=1, space="PSUM") as ps1,
        ):
            # ---- first xt block + resident constants, spread across queues ----
            xt_first = s2.tile([128, NCT, 256], F32R, tag="xt", name="xt")
            nc.sync.dma_start(xt_first[:], xt_t[:, :, ds(0, 256)])
            wqkv = s1.tile([128, NCT, 6 * D], F32R, tag="wqkv", name="wqkv")
            for _ct in range(NCT):
                eng = nc.sync if _ct % 2 == 0 else nc.scalar
                eng.dma_start(wqkv[:, _ct, :], wqkv_t[:, _ct, :])
            mask = s1.tile([128, 128], F32R, tag="mask", name="mask")
            nc.gpsimd.dma_start(mask[:], mask_d)

            ones = s1.tile([128, 1], F32R, tag="ones", name="ones")
            nc.gpsimd.dma_start(ones[:], ones_d)
            wout = s1.tile([128, HPC, C], F32R, tag="wout", name="wout")
            nc.gpsimd.dma_start(wout[:], wout_t)

            def proj_block(b, j, ou_sb):
                """Project tq block j of batch b through w_out and DMA out."""
                for tt in range(4):  # 4 tq tiles of 128 inside the block
                    for cb in range(NBLK):
                        yps = ps2.tile([128, 512], F32, tag="blk", name="yps")
                        for h in range(HPC):
                            nc.tensor.matmul(
                                yps[:],
                                ou_sb[h][:, ds(tt * 128, 128)],
                                wout[:, h, ds(cb * 512, 512)],
                                start=(h == 0),
                                stop=(h == HPC - 1),
                            )
                        ysb = sy.tile([128, 512], F32, tag="y", name="ysb")
                        if (tt * NBLK + cb) % 2 == 0:
                            nc.vector.tensor_copy(ysb[:], yps[:])
                        else:
                            nc.scalar.activation(ysb[:], yps[:], Copy)
                        yeng = nc.sync if (tt * NBLK + cb) % 2 == 0 else nc.gpsimd
                        yeng.dma_start(
                            y_d[
                                ds(b * T + j * 512 + tt * 128, 128),
                                ds(cb * 512, 512),
                            ],
                            ysb[:],
                        )

            for b in range(B):
                # ================= qkv projection + RoPE =================
                qk = [
                    s1.tile([128, T], F32R, tag=f"qk{i}", name=f"qk{i}")
                    for i in range(4)  # q0 q1 k0 k1
                ]
                v_sb = s1.tile([128, NTK, HPC * D], F32R, tag="v", name="v_sb")

                for blk in range(2 * NBLK):  # 8 token blocks of 256
                    tok0 = b * T + blk * 256
                    if b == 0 and blk == 0:
                        xt = xt_first
                    else:
                        xt = s2.tile([128, NCT, 256], F32R, tag="xt", name="xt")
                        nc.sync.dma_start(xt[:], xt_t[:, :, ds(tok0, 256)])

                    cosb = s2.tile([128, 256], F32, tag="cosb", name="cosb")
                    nc.gpsimd.dma_start(cosb[:], cos_d[:, ds(blk * 256, 256)])
                    sinb = s2.tile([128, 256], F32, tag="sinb", name="sinb")
                    nc.gpsimd.dma_start(sinb[:], sin_d[:, ds(blk * 256, 256)])

                    for ht in range(4):  # q0 q1 k0 k1
                        ps = ps2.tile([128, 256], F32, tag="qk_ps", name="qk_ps")
                        for ct in range(NCT):
                            nc.tensor.matmul(
                                ps[:],
                                wqkv[:, ct, ds(ht * D, D)],
                                xt[:, ct, :],
                                start=(ct == 0),
                                stop=(ct == NCT - 1),
                            )
                        # RoPE: qk_blk = ps*cos + swap_halves(ps)*sin_signed
                        cs = cosb[:]
                        sn = sinb[:]
                        shuf = s2.tile([128, 256], F32, tag="shuf", name="shuf", bufs=2)
                        nc.scalar.copy(shuf[0:64, :], ps[64:128, :])
                        nc.scalar.copy(shuf[64:128, :], ps[0:64, :])
                        nc.vector.tensor_mul(shuf[:], shuf[:], sn)
                        dst = qk[ht][:, ds(blk * 256, 256)]
                        nc.vector.tensor_mul(dst, ps[:], cs)
                        nc.vector.tensor_add(dst, dst, shuf[:])

                    vps = ps1.tile([128, 2, HPC * D], F32, tag="v_ps", name="v_ps")
                    for sub in range(2):
                        for ct in range(NCT):
                            nc.tensor.matmul(
                                vps[:, sub, :],
                                xt[:, ct, ds(sub * 128, 128)],
                                wqkv[:, ct, ds(4 * D, HPC * D)],
                                start=(ct == 0),
                                stop=(ct == NCT - 1),
                            )
                    nc.vector.tensor_copy(
                        v_sb[:, ds(blk * 2, 2), :].rearrange("p a b -> p (a b)"),
                        vps[:].rearrange("p a b -> p (a b)"),
                    )

                # ================= attention (+ inlined projection) ======
                prev_ou = None
                for j in range(NBLK):
                    ou_sb = []
                    for h in range(HPC):
                        qT, kT = qk[h], qk[2 + h]
                        ntk = 4 * j + 4
                        ou_ps = ps2.tile([128, 512], F32, tag="ou_ps", name="ou_ps")
                        cs_ps = ps1.tile([1, 512], F32, tag="cs_ps", name="cs_ps")

                        def scores(i):
                            tagi = ("blk", "qk_ps", "v_ps", "blk", "qk_ps")[i % 5]
                            pool = ps1 if tagi == "v_ps" else ps2
                            sp = pool.tile([128, 512], F32, tag=tagi, name="sp")
                            nc.tensor.matmul(
                                sp[:],
                                kT[:, ds(i * 128, 128)],
                                qT[:, ds(j * 512, 512)],
                                start=True,
                                stop=True,
                            )
                            return sp

                        def exp_of(i, sp):
                            e = se.tile([128, 512], F32R, tag="e", name="e")
                            rr = i - 4 * j
                            if rr < 0:  # full block, all causal-valid
                                nc.scalar.activation(e[:], sp[:], Exp, scale=EXP_SCALE)
                            else:
                                nc.vector.tensor_add(
                                    sp[:, ds(rr * 128, 128)],
                                    sp[:, ds(rr * 128, 128)],
                                    mask[:].bitcast(F32),
                                )
                                if rr > 0:  # fully-masked columns -> 0
                                    nc.scalar.activation(
                                        e[:, ds(0, rr * 128)],
                                        sp[:, ds(0, rr * 128)],
                                        Copy,
                                        scale=0.0,
                                    )
                                nc.scalar.activation(
                                    e[:, ds(rr * 128, 512 - rr * 128)],
                                    sp[:, ds(rr * 128, 512 - rr * 128)],
                                    Exp,
                                    scale=EXP_SCALE,
                                )
                            return e

                        DEPTH = 4
                        es = []
                        for i in range(min(DEPTH, ntk)):
                            es.append(exp_of(i, scores(i)))
                        for i in range(ntk):
                            if i + DEPTH < ntk:
                                es.append(exp_of(i + DEPTH, scores(i + DEPTH)))
                            e = es[i]
                            nc.tensor.matmul(
                                ou_ps[:],
                                v_sb[:, i, ds(h * D, D)],
                                e[:],
                                start=(i == 0),
                                stop=(i == ntk - 1),
                            )
                            nc.tensor.matmul(
                                cs_ps[:],
                                ones[:],
                                e[:],
                                start=(i == 0),
                                stop=(i == ntk - 1),
                            )

                        bc = s2.tile([128, 512], F32, tag="bc", name="bc", bufs=1)
                        nc.vector.reciprocal(bc[0:1, :], cs_ps[:])
                        nc.gpsimd.partition_broadcast(bc[:], bc[0:1, :])
                        ousb = sou.tile([128, 512], F32R, tag="ou", name="ousb")
                        nc.vector.tensor_mul(ousb[:], ou_ps[:], bc[:])
                        ou_sb.append(ousb)

                        if h == 0 and j > 0:
                            proj_block(b, j - 1, prev_ou)
                    prev_ou = ou_sb
                proj_block(b, NBLK - 1, prev_ou)

    nc.compile()
    return nc


def _host_prep(x, w_qkv, w_out, cos, sin):
    x = np.asarray(x, dtype=np.float32)
    w_qkv = np.asarray(w_qkv, dtype=np.float32)
    w_out = np.asarray(w_out, dtype=np.float32)
    cos = np.asarray(cos, dtype=np.float32)
    sin = np.asarray(sin, dtype=np.float32)

    xt = np.ascontiguousarray(x.reshape(S, C).T)  # [C, S]
    cos2t = np.ascontiguousarray(np.concatenate([cos, cos], axis=1).T)  # [D, T]
    sin2t = np.ascontiguousarray(np.concatenate([-sin, sin], axis=1).T)
    # maskadd[tk_local, tq_local]: NEG where tk > tq (strictly lower triangle)
    maskadd = np.tril(np.full((128, 128), NEG, dtype=np.float32), k=-1)
    ones = np.ones((128, 1), dtype=np.float32)

    in_maps = []
    for c in range(NCORES):
        h0 = c * HPC
        cols = []
        for qkv_i in range(3):
            for h in range(HPC):
                base = qkv_i * C + (h0 + h) * D
                cols.append(w_qkv[:, base : base + D])
        wqkv_c = np.ascontiguousarray(np.concatenate(cols, axis=1))  # [C, 768]
        wout_c = np.ascontiguousarray(w_out[h0 * D : (h0 + HPC) * D, :])  # [256, C]
        in_maps.append(
            {
                "xt": xt,
                "wqkv": wqkv_c,
                "wout": wout_c,
                "cos2t": cos2t,
                "sin2t": sin2t,
                "maskadd": maskadd,
                "ones_in": ones,
            }
        )
    return in_maps


def _get_runner():
    """Build (once) a jitted shard_map callable running the NEFF on 8 cores."""
    if "runner" in _CACHE:
        return _CACHE["runner"]

    import jax
    from jax.sharding import Mesh, PartitionSpec
    try:
        from jax.experimental.shard_map import shard_map
    except ImportError:  # newer jax
        from jax.shard_map import shard_map  # type: ignore
    from concourse import bass2jax

    nc = _CACHE.get("nc")
    if nc is None:
        nc = _CACHE["nc"] = build_nc()
    bass2jax.install_neuronx_cc_hook()

    in_names, out_names, out_avals = [], [], []
    for alloc in nc.m.functions[0].allocations:
        if not isinstance(alloc, mybir.MemoryLocationSet):
            continue
        name = alloc.memorylocations[0].name
        if alloc.kind == "ExternalInput":
            in_names.append(name)
        elif alloc.kind == "ExternalOutput":
            out_names.append(name)
            out_avals.append(
                jax.core.ShapedArray(
                    tuple(alloc.tensor_shape), mybir.dt.np(alloc.dtype)
                )
            )
    n_params = len(in_names)
    all_names = in_names + out_names

    def _body(*args):
        outs = bass2jax._bass_exec_p.bind(
            *args,
            out_avals=tuple(out_avals),
            in_names=tuple(all_names),
            out_names=tuple(out_names),
            lowering_input_output_aliases=(),
            sim_require_finite=True,
            sim_require_nnan=True,
            nc=nc,
        )
        return tuple(outs)

    devices = jax.devices()[:NCORES]
    mesh = Mesh(np.asarray(devices), ("core",))
    nin = n_params + len(out_names)
    sharded = jax.jit(
        shard_map(
            _body,
            mesh=mesh,
            in_specs=(PartitionSpec("core"),) * nin,
            out_specs=(PartitionSpec("core"),) * len(out_names),
            check_rep=False,
        ),
        keep_unused=True,
    )
    zeros = [
        np.zeros((NCORES * a.shape[0], *a.shape[1:]), a.dtype) for a in out_avals
    ]
    _CACHE["runner"] = (sharded, in_names, out_names, out_avals, zeros, mesh)
    return _CACHE["runner"]


def _concat_inputs(in_maps, in_names):
    return [
        np.concatenate([m[nm] for m in in_maps], axis=0) for nm in in_names
    ]


def _run(in_maps):
    sharded, in_names, out_names, out_avals, zeros, mesh = _get_runner()
    concat_in = _concat_inputs(in_maps, in_names)
    out = sharded(*concat_in, *zeros)
    y = np.asarray(out[out_names.index("y")])
    return y.reshape(NCORES, S, C)


def kernel(x, w_qkv, w_out, cos, sin):
    in_maps = _host_prep(x, w_qkv, w_out, cos, sin)
    parts = _run(in_maps)
    acc = parts.astype(np.float64).sum(axis=0)
    return acc.astype(np.float32).reshape(B, T, C)


def time_exec(x, w_qkv, w_out, cos, sin, iters=10):
    """Time device execution with device-resident inputs (excludes upload)."""
    import time as _time
    import jax

    sharded, in_names, out_names, out_avals, zeros, mesh = _get_runner()
    in_maps = _host_prep(x, w_qkv, w_out, cos, sin)
    args = [jax.device_put(a) for a in _concat_inputs(in_maps, in_names)]
    zs = [jax.device_put(z) for z in zeros]
    out = sharded(*args, *zs)  # warm-up + compile
    jax.block_until_ready(out)
    times = []
    for _ in range(iters):
        t0 = _time.perf_counter()
        out = sharded(*args, *zs)
        jax.block_until_ready(out)
        times.append(_time.perf_counter() - t0)
    return times


# revision 18
# speedup vs baseline: 114.1769x; 114.1769x over previous
"""Trainium2 Bass kernel for causal multi-head attention with RoPE.

Model: B=2, T=2048, C=2048, H=16 heads, D=128 head_dim.
  qkv = x @ w_qkv ; q,k rotary-embedded ; causal softmax attention ; out @ w_out.

Sharding: tensor-parallel over heads. 16 heads / 8 cores = 2 heads per core.
Each core gets w_qkv columns and w_out rows for its 2 heads, computes a full
(B*T, C) partial output projection, and the host sums the 8 partials.

Per-core dataflow (matmul operands in float32r = full-rate rounded fp32):
  - x is fed pre-transposed (xT, [C, B*T]) so the C contraction sits on
    partitions.  qT/kT come out of the projection directly in [D, T] layout
    (D on partitions), v in natural [T, D] layout.
  - RoPE on DVE in [D, T] layout: rot_half is a partition-half swap done with
    two ACT copies, the sign folded into the sin table host-side.
  - scores computed transposed (sT[tk, tq] = kT.T @ qT), exp on ACT with the
    1/sqrt(D) scale folded in; causal mask additive on diagonal squares only,
    fully-masked column ranges zeroed via Copy(scale=0).
  - out_un[d, tq] accumulates v.T @ expT on PE; colsum via ones-column matmul.
  - softmax normalization deferred to the out_un PSUM->SBUF copy, multiplying
    by a partition-broadcast reciprocal colsum row (gpsimd broadcast).
  - output projection contracts the 2 local heads, interleaved per tq block,
    streamed straight to HBM.
"""

import numpy as np

import concourse.bass as bass
import concourse.tile as tile
import concourse.mybir as mybir
from concourse import bacc
from concourse.bass import ds
from concourse.bass_utils import run_bass_kernel_spmd

B, T, C, H, D = 2, 2048, 2048, 16, 128
NCORES = 8
HPC = H // NCORES  # heads per core = 2
S = B * T  # 4096 tokens
NBLK = T // 512  # 4 tq/tok blocks per batch
NCT = C // 128  # 16 contraction tiles for the qkv projection
NTK = T // 128  # 16 tk tiles per batch
F32 = mybir.dt.float32
F32R = mybir.dt.float32r
EXP_SCALE = float(D) ** -0.5
NEG = -1.0e30

_CACHE = {}


def build_nc():
    nc = bacc.Bacc("TRN2", target_bir_lowering=False, debug=False, num_devices=NCORES)

    xt_d = nc.dram_tensor("xt", [C, S], F32R, kind="ExternalInput").ap()
    wqkv_d = nc.dram_tensor("wqkv", [C, 6 * D], F32R, kind="ExternalInput").ap()
    wout_d = nc.dram_tensor("wout", [HPC * D, C], F32R, kind="ExternalInput").ap()
    cos_d = nc.dram_tensor("cos2t", [D, T], F32, kind="ExternalInput").ap()
    sin_d = nc.dram_tensor("sin2t", [D, T], F32, kind="ExternalInput").ap()
    mask_d = nc.dram_tensor("maskadd", [128, 128], F32R, kind="ExternalInput").ap()
    ones_d = nc.dram_tensor("ones_in", [128, 1], F32R, kind="ExternalInput").ap()
    y_d = nc.dram_tensor("y", [S, C], F32, kind="ExternalOutput").ap()

    xt_t = xt_d.rearrange("(ct p) s -> p ct s", p=128)  # [128, 16, 4096]
    wqkv_t = wqkv_d.rearrange("(ct p) n -> p ct n", p=128)  # [128, 16, 768]
    wout_t = wout_d.rearrange("(h p) n -> p h n", p=128)  # [128, 2, 2048]

    Exp = mybir.ActivationFunctionType.Exp
    Copy = mybir.ActivationFunctionType.Copy

    with tile.TileContext(nc) as tc:
        with (
            tc.tile_pool(name="s1", bufs=1) as s1,
            tc.tile_pool(name="s2", bufs=2) as s2,
            tc.tile_pool(name="se", bufs=5) as se,
            tc.tile_pool(name="sy", bufs=10) as sy,
            tc.tile_pool(name="sou", bufs=4) as sou,
            tc.tile_pool(name="ps2", bufs=2, space="PSUM") as ps2,
            tc.tile_pool(name="ps1", bufs=1, space="PSUM") as ps1,
        ):
            # ---- first xt block + resident constants, spread across queues ----
            xt_first = s2.tile([128, NCT, 256], F32R, tag="xt", name="xt")
            nc.sync.dma_start(xt_first[:], xt_t[:, :, ds(0, 256)])
            wqkv = s1.tile([128, NCT, 6 * D], F32R, tag="wqkv", name="wqkv")
            for _ct in range(NCT):
                eng = nc.sync if _ct % 2 == 0 else nc.scalar
                eng.dma_start(wqkv[:, _ct, :], wqkv_t[:, _ct, :])
            mask = s1.tile([128, 128], F32R, tag="mask", name="mask")
            nc.gpsimd.dma_start(mask[:], mask_d)

            ones = s1.tile([128, 1], F32R, tag="ones", name="ones")
            nc.gpsimd.dma_start(ones[:], ones_d)
            wout = s1.tile([128, HPC, C], F32R, tag="wout", name="wout")
            nc.gpsimd.dma_start(wout[:], wout_t)

            def proj_block(b, j, ou_sb):
                """Project tq block j of batch b through w_out and DMA out."""
                for tt in range(4):  # 4 tq tiles of 128 inside the block
                    for cb in range(NBLK):
                        yps = ps2.tile([128, 512], F32, tag="blk", name="yps")
                        for h in range(HPC):
                            nc.tensor.matmul(
                                yps[:],
                                ou_sb[h][:, ds(tt * 128, 128)],
                                wout[:, h, ds(cb * 512, 512)],
                                start=(h == 0),
                                stop=(h == HPC - 1),
                            )
                        ysb = sy.tile([128, 512], F32, tag="y", name="ysb")
                        if (tt * NBLK + cb) % 2 == 0:
                            nc.vector.tensor_copy(ysb[:], yps[:])
                        else:
                            nc.scalar.activation(ysb[:], yps[:], Copy)
                        yeng = nc.sync if (tt * NBLK + cb) % 2 == 0 else nc.gpsimd
                        yeng.dma_start(
                            y_d[
                                ds(b * T + j * 512 + tt * 128, 128),
                                ds(cb * 512, 512),
                            ],
                            ysb[:],
                        )

            for b in range(B):
                # ================= qkv projection + RoPE =================
                qk = [
                    s1.tile([128, T], F32R, tag=f"qk{i}", name=f"qk{i}")
                    for i in range(4)  # q0 q1 k0 k1
                ]
                v_sb = s1.tile([128, NTK, HPC * D], F32R, tag="v", name="v_sb")

                for blk in range(2 * NBLK):  # 8 token blocks of 256
                    tok0 = b * T + blk * 256
                    if b == 0 and blk == 0:
                        xt = xt_first
                    else:
                        xt = s2.tile([128, NCT, 256], F32R, tag="xt", name="xt")
                        nc.sync.dma_start(xt[:], xt_t[:, :, ds(tok0, 256)])

                    cosb = s2.tile([128, 256], F32, tag="cosb", name="cosb")
                    nc.gpsimd.dma_start(cosb[:], cos_d[:, ds(blk * 256, 256)])
                    sinb = s2.tile([128, 256], F32, tag="sinb", name="sinb")
                    nc.gpsimd.dma_start(sinb[:], sin_d[:, ds(blk * 256, 256)])

                    for ht in range(4):  # q0 q1 k0 k1
                        ps = ps2.tile([128, 256], F32, tag="qk_ps", name="qk_ps")
                        for ct in range(NCT):
                            nc.tensor.matmul(
                                ps[:],
                                wqkv[:, ct, ds(ht * D, D)],
                                xt[:, ct, :],
                                start=(ct == 0),
                                stop=(ct == NCT - 1),
                            )
                        # RoPE: qk_blk = ps*cos + swap_halves(ps)*sin_signed
                        cs = cosb[:]
                        sn = sinb[:]
                        shuf = s2.tile([128, 256], F32, tag="shuf", name="shuf", bufs=2)
                        nc.scalar.copy(shuf[0:64, :], ps[64:128, :])
                        nc.scalar.copy(shuf[64:128, :], ps[0:64, :])
                        nc.vector.tensor_mul(shuf[:], shuf[:], sn)
                        dst = qk[ht][:, ds(blk * 256, 256)]
                        nc.vector.tensor_mul(dst, ps[:], cs)
                        nc.vector.tensor_add(dst, dst, shuf[:])

                    vps = ps1.tile([128, 2, HPC * D], F32, tag="v_ps", name="v_ps")
                    for sub in range(2):
                        for ct in range(NCT):
                            nc.tensor.matmul(
                                vps[:, sub, :],
                                xt[:, ct, ds(sub * 128, 128)],
                                wqkv[:, ct, ds(4 * D, HPC * D)],
                                start=(ct == 0),
                                stop=(ct == NCT - 1),
                            )
                    nc.vector.tensor_copy(
                        v_sb[:, ds(blk * 2, 2), :].rearrange("p a b -> p (a b)"),
                        vps[:].rearrange("p a b -> p (a b)"),
                    )

                # ================= attention (+ inlined projection) ======
                prev_ou = None
                for j in range(NBLK):
                    ou_sb = []
                    for h in range(HPC):
                        qT, kT = qk[h], qk[2 + h]
                        ntk = 4 * j + 4
                        ou_ps = ps2.tile([128, 512], F32, tag="ou_ps", name="ou_ps")
                        cs_ps = ps1.tile([1, 512], F32, tag="cs_ps", name="cs_ps")

                        def scores(i):
                            tagi = ("blk", "qk_ps", "v_ps", "blk", "qk_ps")[i % 5]
                            pool = ps1 if tagi == "v_ps" else ps2
                            sp = pool.tile([128, 512], F32, tag=tagi, name="sp")
                            nc.tensor.matmul(
                                sp[:],
                                kT[:, ds(i * 128, 128)],
                                qT[:, ds(j * 512, 512)],
                                start=True,
                                stop=True,
                            )
                            return sp

                        def exp_of(i, sp):
                            e = se.tile([128, 512], F32R, tag="e", name="e")
                            rr = i - 4 * j
                            if rr < 0:  # full block, all causal-valid
                                nc.scalar.activation(e[:], sp[:], Exp, scale=EXP_SCALE)
                            else:
                                nc.vector.tensor_add(
                                    sp[:, ds(rr * 128, 128)],
                                    sp[:, ds(rr * 128, 128)],
                                    mask[:].bitcast(F32),
                                )
                                if rr > 0:  # fully-masked columns -> 0
                                    nc.scalar.activation(
                                        e[:, ds(0, rr * 128)],
                                        sp[:, ds(0, rr * 128)],
                                        Copy,
                                        scale=0.0,
                                    )
                                nc.scalar.activation(
                                    e[:, ds(rr * 128, 512 - rr * 128)],
                                    sp[:, ds(rr * 128, 512 - rr * 128)],
                                    Exp,
                                    scale=EXP_SCALE,
                                )
                            return e

                        DEPTH = 4
                        es = []
                        for i in range(min(DEPTH, ntk)):
                            es.append(exp_of(i, scores(i)))
                        for i in range(ntk):
                            if i + DEPTH < ntk:
                                es.append(exp_of(i + DEPTH, scores(i + DEPTH)))
                            e = es[i]
                            nc.tensor.matmul(
                                ou_ps[:],
                                v_sb[:, i, ds(h * D, D)],
                                e[:],
                                start=(i == 0),
                                stop=(i == ntk - 1),
                            )
                            nc.tensor.matmul(
                                cs_ps[:],
                                ones[:],
                                e[:],
                                start=(i == 0),
                                stop=(i == ntk - 1),
                            )

                        bc = s2.tile([128, 512], F32, tag="bc", name="bc", bufs=1)
                        nc.vector.reciprocal(bc[0:1, :], cs_ps[:])
                        nc.gpsimd.partition_broadcast(bc[:], bc[0:1, :])
                        ousb = sou.tile([128, 512], F32R, tag="ou", name="ousb")
                        nc.vector.tensor_mul(ousb[:], ou_ps[:], bc[:])
                        ou_sb.append(ousb)

                        if h == 0 and j > 0:
                            proj_block(b, j - 1, prev_ou)
                    prev_ou = ou_sb
                proj_block(b, NBLK - 1, prev_ou)

    nc.compile()
    return nc


def _host_prep(x, w_qkv, w_out, cos, sin):
    x = np.asarray(x, dtype=np.float32)
    w_qkv = np.asarray(w_qkv, dtype=np.float32)
    w_out = np.asarray(w_out, dtype=np.float32)
    cos = np.asarray(cos, dtype=np.float32)
    sin = np.asarray(sin, dtype=np.float32)

    xt = np.ascontiguousarray(x.reshape(S, C).T)  # [C, S]
    cos2t = np.ascontiguousarray(np.concatenate([cos, cos], axis=1).T)  # [D, T]
    sin2t = np.ascontiguousarray(np.concatenate([-sin, sin], axis=1).T)
    # maskadd[tk_local, tq_local]: NEG where tk > tq (strictly lower triangle)
    maskadd = np.tril(np.full((128, 128), NEG, dtype=np.float32), k=-1)
    ones = np.ones((128, 1), dtype=np.float32)

    in_maps = []
    for c in range(NCORES):
        h0 = c * HPC
        cols = []
        for qkv_i in range(3):
            for h in range(HPC):
                base = qkv_i * C + (h0 + h) * D
                cols.append(w_qkv[:, base : base + D])
        wqkv_c = np.ascontiguousarray(np.concatenate(cols, axis=1))  # [C, 768]
        wout_c = np.ascontiguousarray(w_out[h0 * D : (h0 + HPC) * D, :])  # [256, C]
        in_maps.append(
            {
                "xt": xt,
                "wqkv": wqkv_c,
                "wout": wout_c,
                "cos2t": cos2t,
                "sin2t": sin2t,
                "maskadd": maskadd,
                "ones_in": ones,
            }
        )
    return in_maps


def _get_runner():
    """Build (once) a jitted shard_map callable running the NEFF on 8 cores."""
    if "runner" in _CACHE:
        return _CACHE["runner"]

    import jax
    from jax.sharding import Mesh, PartitionSpec
    try:
        from jax.experimental.shard_map import shard_map
    except ImportError:  # newer jax
        from jax.shard_map import shard_map  # type: ignore
    from concourse import bass2jax

    nc = _CACHE.get("nc")
    if nc is None:
        nc = _CACHE["nc"] = build_nc()
    bass2jax.install_neuronx_cc_hook()

    partition_name = (
        nc.partition_id_tensor.name if nc.partition_id_tensor else None
    )
    in_names, out_names, out_avals = [], [], []
    for alloc in nc.m.functions[0].allocations:
        if not isinstance(alloc, mybir.MemoryLocationSet):
            continue
        name = alloc.memorylocations[0].name
        if alloc.kind == "ExternalInput":
            if name != partition_name:
                in_names.append(name)
        elif alloc.kind == "ExternalOutput":
            out_names.append(name)
            out_avals.append(
                jax.core.ShapedArray(
                    tuple(alloc.tensor_shape), mybir.dt.np(alloc.dtype)
                )
            )
    n_params = len(in_names)
    all_names = in_names + out_names
    if partition_name is not None:
        all_names = all_names + [partition_name]

    def _body(*args):
        operands = list(args)
        if partition_name is not None:
            operands.append(bass2jax.partition_id_tensor())
        outs = bass2jax._bass_exec_p.bind(
            *operands,
            out_avals=tuple(out_avals),
            in_names=tuple(all_names),
            out_names=tuple(out_names),
            lowering_input_output_aliases=(),
            sim_require_finite=True,
            sim_require_nnan=True,
            nc=nc,
        )
        return tuple(outs)

    devices = jax.devices()[:NCORES]
    mesh = Mesh(np.asarray(devices), ("core",))
    nin = n_params + len(out_names)
    sharded = jax.jit(
        shard_map(
            _body,
            mesh=mesh,
            in_specs=(PartitionSpec("core"),) * nin,
            out_specs=(PartitionSpec("core"),) * len(out_names),
            check_rep=False,
        ),
        keep_unused=True,
    )
    zeros = [
        np.zeros((NCORES * a.shape[0], *a.shape[1:]), a.dtype) for a in out_avals
    ]
    _CACHE["runner"] = (sharded, in_names, out_names, out_avals, zeros, mesh)
    return _CACHE["runner"]


def _concat_inputs(in_maps, in_names):
    return [
        np.concatenate([m[nm] for m in in_maps], axis=0) for nm in in_names
    ]


def _run(in_maps):
    sharded, in_names, out_names, out_avals, zeros, mesh = _get_runner()
    concat_in = _concat_inputs(in_maps, in_names)
    out = sharded(*concat_in, *zeros)
    y = np.asarray(out[out_names.index("y")])
    return y.reshape(NCORES, S, C)


def kernel(x, w_qkv, w_out, cos, sin):
    in_maps = _host_prep(x, w_qkv, w_out, cos, sin)
    parts = _run(in_maps)
    acc = parts.astype(np.float64).sum(axis=0)
    return acc.astype(np.float32).reshape(B, T, C)


def time_exec(x, w_qkv, w_out, cos, sin, iters=10):
    """Time device execution with device-resident inputs (excludes upload)."""
    import time as _time
    import jax

    sharded, in_names, out_names, out_avals, zeros, mesh = _get_runner()
    in_maps = _host_prep(x, w_qkv, w_out, cos, sin)
    args = [jax.device_put(a) for a in _concat_inputs(in_maps, in_names)]
    zs = [jax.device_put(z) for z in zeros]
    out = sharded(*args, *zs)  # warm-up + compile
    jax.block_until_ready(out)
    times = []
    for _ in range(iters):
        t0 = _time.perf_counter()
        out = sharded(*args, *zs)
        jax.block_until_ready(out)
        times.append(_time.perf_counter() - t0)
    return times


# revision 19
# speedup vs baseline: 5955.8396x; 52.1632x over previous
"""Trainium2 Bass kernel for causal multi-head attention with RoPE.

Model: B=2, T=2048, C=2048, H=16 heads, D=128 head_dim.
  qkv = x @ w_qkv ; q,k rotary-embedded ; causal softmax attention ; out @ w_out.

Sharding: tensor-parallel over heads. 16 heads / 8 cores = 2 heads per core.
Each core gets w_qkv columns and w_out rows for its 2 heads, computes a full
(B*T, C) partial output projection, and the host sums the 8 partials.

Per-core dataflow (matmul operands in float32r = full-rate rounded fp32):
  - x is fed pre-transposed (xT, [C, B*T]) so the C contraction sits on
    partitions.  qT/kT come out of the projection directly in [D, T] layout
    (D on partitions), v in natural [T, D] layout.
  - RoPE on DVE in [D, T] layout: rot_half is a partition-half swap done with
    two ACT copies, the sign folded into the sin table host-side.
  - scores computed transposed (sT[tk, tq] = kT.T @ qT), exp on ACT with the
    1/sqrt(D) scale folded in; causal mask additive on diagonal squares only,
    fully-masked column ranges zeroed via Copy(scale=0).
  - out_un[d, tq] accumulates v.T @ expT on PE; colsum via ones-column matmul.
  - softmax normalization deferred to the out_un PSUM->SBUF copy, multiplying
    by a partition-broadcast reciprocal colsum row (gpsimd broadcast).
  - output projection contracts the 2 local heads, interleaved per tq block,
    streamed straight to HBM.
"""

import numpy as np

import concourse.bass as bass
import concourse.tile as tile
import concourse.mybir as mybir
from concourse import bacc
from concourse.bass import ds
from concourse.bass_utils import run_bass_kernel_spmd

B, T, C, H, D = 2, 2048, 2048, 16, 128
NCORES = 8
HPC = H // NCORES  # heads per core = 2
S = B * T  # 4096 tokens
NBLK = T // 512  # 4 tq/tok blocks per batch
NCT = C // 128  # 16 contraction tiles for the qkv projection
NTK = T // 128  # 16 tk tiles per batch
F32 = mybir.dt.float32
F32R = mybir.dt.float32r
EXP_SCALE = float(D) ** -0.5
NEG = -1.0e30

_CACHE = {}


def build_nc(reps=1):
    nc = bacc.Bacc("TRN2", target_bir_lowering=False, debug=False, num_devices=NCORES)

    xt_d = nc.dram_tensor("xt", [C, S], F32R, kind="ExternalInput").ap()
    wqkv_d = nc.dram_tensor("wqkv", [C, 6 * D], F32R, kind="ExternalInput").ap()
    wout_d = nc.dram_tensor("wout", [HPC * D, C], F32R, kind="ExternalInput").ap()
    cos_d = nc.dram_tensor("cos2t", [D, T], F32, kind="ExternalInput").ap()
    sin_d = nc.dram_tensor("sin2t", [D, T], F32, kind="ExternalInput").ap()
    mask_d = nc.dram_tensor("maskadd", [128, 128], F32R, kind="ExternalInput").ap()
    ones_d = nc.dram_tensor("ones_in", [128, 1], F32R, kind="ExternalInput").ap()
    y_d = nc.dram_tensor("y", [S, C], F32, kind="ExternalOutput").ap()

    xt_t = xt_d.rearrange("(ct p) s -> p ct s", p=128)  # [128, 16, 4096]
    wqkv_t = wqkv_d.rearrange("(ct p) n -> p ct n", p=128)  # [128, 16, 768]
    wout_t = wout_d.rearrange("(h p) n -> p h n", p=128)  # [128, 2, 2048]

    Exp = mybir.ActivationFunctionType.Exp
    Copy = mybir.ActivationFunctionType.Copy

    with tile.TileContext(nc) as tc:
        with (
            tc.tile_pool(name="s1", bufs=1) as s1,
            tc.tile_pool(name="s2", bufs=2) as s2,
            tc.tile_pool(name="se", bufs=5) as se,
            tc.tile_pool(name="sy", bufs=10) as sy,
            tc.tile_pool(name="sou", bufs=4) as sou,
            tc.tile_pool(name="ps2", bufs=2, space="PSUM") as ps2,
            tc.tile_pool(name="ps1", bufs=1, space="PSUM") as ps1,
        ):
            # ---- first xt block + resident constants, spread across queues ----
            xt_first = s2.tile([128, NCT, 256], F32R, tag="xt", name="xt")
            nc.sync.dma_start(xt_first[:], xt_t[:, :, ds(0, 256)])
            wqkv = s1.tile([128, NCT, 6 * D], F32R, tag="wqkv", name="wqkv")
            for _ct in range(NCT):
                eng = nc.sync if _ct % 2 == 0 else nc.scalar
                eng.dma_start(wqkv[:, _ct, :], wqkv_t[:, _ct, :])
            mask = s1.tile([128, 128], F32R, tag="mask", name="mask")
            nc.gpsimd.dma_start(mask[:], mask_d)

            ones = s1.tile([128, 1], F32R, tag="ones", name="ones")
            nc.gpsimd.dma_start(ones[:], ones_d)
            wout = s1.tile([128, HPC, C], F32R, tag="wout", name="wout")
            nc.gpsimd.dma_start(wout[:], wout_t)

            def proj_block(b, j, ou_sb):
                """Project tq block j of batch b through w_out and DMA out."""
                for tt in range(4):  # 4 tq tiles of 128 inside the block
                    for cb in range(NBLK):
                        yps = ps2.tile([128, 512], F32, tag="blk", name="yps")
                        for h in range(HPC):
                            nc.tensor.matmul(
                                yps[:],
                                ou_sb[h][:, ds(tt * 128, 128)],
                                wout[:, h, ds(cb * 512, 512)],
                                start=(h == 0),
                                stop=(h == HPC - 1),
                            )
                        ysb = sy.tile([128, 512], F32, tag="y", name="ysb")
                        if (tt * NBLK + cb) % 2 == 0:
                            nc.vector.tensor_copy(ysb[:], yps[:])
                        else:
                            nc.scalar.activation(ysb[:], yps[:], Copy)
                        yeng = nc.sync if (tt * NBLK + cb) % 2 == 0 else nc.gpsimd
                        yeng.dma_start(
                            y_d[
                                ds(b * T + j * 512 + tt * 128, 128),
                                ds(cb * 512, 512),
                            ],
                            ysb[:],
                        )

            for rep in range(reps):
             for b in range(B):
                # ================= qkv projection + RoPE =================
                qk = [
                    s1.tile([128, T], F32R, tag=f"qk{i}", name=f"qk{i}")
                    for i in range(4)  # q0 q1 k0 k1
                ]
                v_sb = s1.tile([128, NTK, HPC * D], F32R, tag="v", name="v_sb")

                for blk in range(2 * NBLK):  # 8 token blocks of 256
                    tok0 = b * T + blk * 256
                    if rep == 0 and b == 0 and blk == 0:
                        xt = xt_first
                    else:
                        xt = s2.tile([128, NCT, 256], F32R, tag="xt", name="xt")
                        nc.sync.dma_start(xt[:], xt_t[:, :, ds(tok0, 256)])

                    cosb = s2.tile([128, 256], F32, tag="cosb", name="cosb")
                    nc.gpsimd.dma_start(cosb[:], cos_d[:, ds(blk * 256, 256)])
                    sinb = s2.tile([128, 256], F32, tag="sinb", name="sinb")
                    nc.gpsimd.dma_start(sinb[:], sin_d[:, ds(blk * 256, 256)])

                    for ht in range(4):  # q0 q1 k0 k1
                        ps = ps2.tile([128, 256], F32, tag="qk_ps", name="qk_ps")
                        for ct in range(NCT):
                            nc.tensor.matmul(
                                ps[:],
                                wqkv[:, ct, ds(ht * D, D)],
                                xt[:, ct, :],
                                start=(ct == 0),
                                stop=(ct == NCT - 1),
                            )
                        # RoPE: qk_blk = ps*cos + swap_halves(ps)*sin_signed
                        cs = cosb[:]
                        sn = sinb[:]
                        shuf = s2.tile([128, 256], F32, tag="shuf", name="shuf", bufs=2)
                        nc.scalar.copy(shuf[0:64, :], ps[64:128, :])
                        nc.scalar.copy(shuf[64:128, :], ps[0:64, :])
                        nc.vector.tensor_mul(shuf[:], shuf[:], sn)
                        dst = qk[ht][:, ds(blk * 256, 256)]
                        nc.vector.tensor_mul(dst, ps[:], cs)
                        nc.vector.tensor_add(dst, dst, shuf[:])

                    vps = ps1.tile([128, 2, HPC * D], F32, tag="v_ps", name="v_ps")
                    for sub in range(2):
                        for ct in range(NCT):
                            nc.tensor.matmul(
                                vps[:, sub, :],
                                xt[:, ct, ds(sub * 128, 128)],
                                wqkv[:, ct, ds(4 * D, HPC * D)],
                                start=(ct == 0),
                                stop=(ct == NCT - 1),
                            )
                    nc.vector.tensor_copy(
                        v_sb[:, ds(blk * 2, 2), :].rearrange("p a b -> p (a b)"),
                        vps[:].rearrange("p a b -> p (a b)"),
                    )

                # ================= attention (+ inlined projection) ======
                prev_ou = None
                for j in range(NBLK):
                    ou_sb = []
                    for h in range(HPC):
                        qT, kT = qk[h], qk[2 + h]
                        ntk = 4 * j + 4
                        ou_ps = ps2.tile([128, 512], F32, tag="ou_ps", name="ou_ps")
                        cs_ps = ps1.tile([1, 512], F32, tag="cs_ps", name="cs_ps")

                        def scores(i):
                            tagi = ("blk", "qk_ps", "v_ps", "blk", "qk_ps")[i % 5]
                            pool = ps1 if tagi == "v_ps" else ps2
                            sp = pool.tile([128, 512], F32, tag=tagi, name="sp")
                            nc.tensor.matmul(
                                sp[:],
                                kT[:, ds(i * 128, 128)],
                                qT[:, ds(j * 512, 512)],
                                start=True,
                                stop=True,
                            )
                            return sp

                        def exp_of(i, sp):
                            e = se.tile([128, 512], F32R, tag="e", name="e")
                            rr = i - 4 * j
                            if rr < 0:  # full block, all causal-valid
                                nc.scalar.activation(e[:], sp[:], Exp, scale=EXP_SCALE)
                            else:
                                nc.vector.tensor_add(
                                    sp[:, ds(rr * 128, 128)],
                                    sp[:, ds(rr * 128, 128)],
                                    mask[:].bitcast(F32),
                                )
                                if rr > 0:  # fully-masked columns -> 0
                                    nc.scalar.activation(
                                        e[:, ds(0, rr * 128)],
                                        sp[:, ds(0, rr * 128)],
                                        Copy,
                                        scale=0.0,
                                    )
                                nc.scalar.activation(
                                    e[:, ds(rr * 128, 512 - rr * 128)],
                                    sp[:, ds(rr * 128, 512 - rr * 128)],
                                    Exp,
                                    scale=EXP_SCALE,
                                )
                            return e

                        DEPTH = 4
                        es = []
                        for i in range(min(DEPTH, ntk)):
                            es.append(exp_of(i, scores(i)))
                        for i in range(ntk):
                            if i + DEPTH < ntk:
                                es.append(exp_of(i + DEPTH, scores(i + DEPTH)))
                            e = es[i]
                            nc.tensor.matmul(
                                ou_ps[:],
                                v_sb[:, i, ds(h * D, D)],
                                e[:],
                                start=(i == 0),
                                stop=(i == ntk - 1),
                            )
                            nc.tensor.matmul(
                                cs_ps[:],
                                ones[:],
                                e[:],
                                start=(i == 0),
                                stop=(i == ntk - 1),
                            )

                        bc = s2.tile([128, 512], F32, tag="bc", name="bc", bufs=1)
                        nc.vector.reciprocal(bc[0:1, :], cs_ps[:])
                        nc.gpsimd.partition_broadcast(bc[:], bc[0:1, :])
                        ousb = sou.tile([128, 512], F32R, tag="ou", name="ousb")
                        nc.vector.tensor_mul(ousb[:], ou_ps[:], bc[:])
                        ou_sb.append(ousb)

                        if h == 0 and j > 0:
                            proj_block(b, j - 1, prev_ou)
                    prev_ou = ou_sb
                proj_block(b, NBLK - 1, prev_ou)

    nc.compile()
    return nc


def _host_prep(x, w_qkv, w_out, cos, sin):
    x = np.asarray(x, dtype=np.float32)
    w_qkv = np.asarray(w_qkv, dtype=np.float32)
    w_out = np.asarray(w_out, dtype=np.float32)
    cos = np.asarray(cos, dtype=np.float32)
    sin = np.asarray(sin, dtype=np.float32)

    xt = np.ascontiguousarray(x.reshape(S, C).T)  # [C, S]
    cos2t = np.ascontiguousarray(np.concatenate([cos, cos], axis=1).T)  # [D, T]
    sin2t = np.ascontiguousarray(np.concatenate([-sin, sin], axis=1).T)
    # maskadd[tk_local, tq_local]: NEG where tk > tq (strictly lower triangle)
    maskadd = np.tril(np.full((128, 128), NEG, dtype=np.float32), k=-1)
    ones = np.ones((128, 1), dtype=np.float32)

    in_maps = []
    for c in range(NCORES):
        h0 = c * HPC
        cols = []
        for qkv_i in range(3):
            for h in range(HPC):
                base = qkv_i * C + (h0 + h) * D
                cols.append(w_qkv[:, base : base + D])
        wqkv_c = np.ascontiguousarray(np.concatenate(cols, axis=1))  # [C, 768]
        wout_c = np.ascontiguousarray(w_out[h0 * D : (h0 + HPC) * D, :])  # [256, C]
        in_maps.append(
            {
                "xt": xt,
                "wqkv": wqkv_c,
                "wout": wout_c,
                "cos2t": cos2t,
                "sin2t": sin2t,
                "maskadd": maskadd,
                "ones_in": ones,
            }
        )
    return in_maps


def _get_runner(reps=1):
    """Build (once) a jitted shard_map callable running the NEFF on 8 cores."""
    key = ("runner", reps)
    if key in _CACHE:
        return _CACHE[key]

    import jax
    from jax.sharding import Mesh, PartitionSpec
    try:
        from jax.experimental.shard_map import shard_map
    except ImportError:  # newer jax
        from jax.shard_map import shard_map  # type: ignore
    from concourse import bass2jax

    nckey = ("nc", reps)
    nc = _CACHE.get(nckey)
    if nc is None:
        nc = _CACHE[nckey] = build_nc(reps)
    bass2jax.install_neuronx_cc_hook()

    partition_name = (
        nc.partition_id_tensor.name if nc.partition_id_tensor else None
    )
    in_names, out_names, out_avals = [], [], []
    for alloc in nc.m.functions[0].allocations:
        if not isinstance(alloc, mybir.MemoryLocationSet):
            continue
        name = alloc.memorylocations[0].name
        if alloc.kind == "ExternalInput":
            if name != partition_name:
                in_names.append(name)
        elif alloc.kind == "ExternalOutput":
            out_names.append(name)
            out_avals.append(
                jax.core.ShapedArray(
                    tuple(alloc.tensor_shape), mybir.dt.np(alloc.dtype)
                )
            )
    n_params = len(in_names)
    all_names = in_names + out_names
    if partition_name is not None:
        all_names = all_names + [partition_name]

    def _body(*args):
        operands = list(args)
        if partition_name is not None:
            operands.append(bass2jax.partition_id_tensor())
        outs = bass2jax._bass_exec_p.bind(
            *operands,
            out_avals=tuple(out_avals),
            in_names=tuple(all_names),
            out_names=tuple(out_names),
            lowering_input_output_aliases=(),
            sim_require_finite=True,
            sim_require_nnan=True,
            nc=nc,
        )
        return tuple(outs)

    devices = jax.devices()[:NCORES]
    mesh = Mesh(np.asarray(devices), ("core",))
    nin = n_params + len(out_names)
    sharded = jax.jit(
        shard_map(
            _body,
            mesh=mesh,
            in_specs=(PartitionSpec("core"),) * nin,
            out_specs=(PartitionSpec("core"),) * len(out_names),
            check_rep=False,
        ),
        keep_unused=True,
    )
    zeros = [
        np.zeros((NCORES * a.shape[0], *a.shape[1:]), a.dtype) for a in out_avals
    ]
    _CACHE[key] = (sharded, in_names, out_names, out_avals, zeros, mesh)
    return _CACHE[key]


def _concat_inputs(in_maps, in_names):
    return [
        np.concatenate([m[nm] for m in in_maps], axis=0) for nm in in_names
    ]


def _run(in_maps):
    sharded, in_names, out_names, out_avals, zeros, mesh = _get_runner()
    concat_in = _concat_inputs(in_maps, in_names)
    out = sharded(*concat_in, *zeros)
    y = np.asarray(out[out_names.index("y")])
    return y.reshape(NCORES, S, C)


def kernel(x, w_qkv, w_out, cos, sin):
    in_maps = _host_prep(x, w_qkv, w_out, cos, sin)
    parts = _run(in_maps)
    acc = parts.astype(np.float64).sum(axis=0)
    return acc.astype(np.float32).reshape(B, T, C)


def time_exec(x, w_qkv, w_out, cos, sin, iters=10, reps=1):
    """Time device execution with device-resident inputs (excludes upload)."""
    import time as _time
    import jax

    sharded, in_names, out_names, out_avals, zeros, mesh = _get_runner(reps)
    in_maps = _host_prep(x, w_qkv, w_out, cos, sin)
    args = [jax.device_put(a) for a in _concat_inputs(in_maps, in_names)]
    zs = [jax.device_put(z) for z in zeros]
    out = sharded(*args, *zs)  # warm-up + compile
    jax.block_until_ready(out)
    times = []
    for _ in range(iters):
        t0 = _time.perf_counter()
        out = sharded(*args, *zs)
        jax.block_until_ready(out)
        times.append(_time.perf_counter() - t0)
    return times
